# revision 12
# baseline (speedup 1.0000x reference)
"""Soft-DTW forward (gamma=1) Bass kernel for Trainium2, 8 NeuronCores.

Algorithm
---------
Exp-domain DP: with E = exp(-R/gamma), the soft-min recurrence becomes
    E[i,j] = exp(-D[i,j]) * (E[i-1,j] + E[i-1,j-1] + E[i,j-1])
(pure multiply-add, no per-cell exp/log).  Dynamic range is handled with a
drift factor:  F[i,j] = E[i,j] * exp(beta*(i+j)),  beta ~ E[dR/dstep], giving
    F[i,j] = et[i,j] * (F[i-1,j] + wb*F[i-1,j-1] + F[i,j-1])
with et = exp(beta - D), wb = exp(beta).  Per row this is a first-order linear
recurrence along j, which maps onto the DVE tensor_tensor_scan:
    state = (A_j + state) * et_j,   A_j = F_up[j] + wb*F_diag[j-1]
i.e. 2 DVE instructions per row-step (one scalar_tensor_tensor + one scan).

Parallelization: batch is sharded 16 per core (data parallel).  On a core, the
512x512 DP is split into fwd (rows 1..256 from the top-left) and bwd (rows
511..256 on the reversed matrix) halves, joined through a seam formula on the
two middle rows (soft-DTW path sums factor across the seam).  Each direction's
row is split into 4 column groups of width 128, software-pipelined in a
wavefront (group g processes row t-g at macro-step t), so every step is a
full 128-partition x 128-element instruction:
    partition p = (3-g)*32 + dir*16 + b  (b = batch-in-core, g = col group)
Cross-group carries move by a 1-partition-offset copy on GPSIMD; the scan's
slot-0 "passthrough" column (A=0, et=1) regenerates each group's diagonal halo
without extra copies, and et's slot-0 value (0 for border groups) kills
garbage carries at column borders.

The host pre-skews D into the exact staging layout (33 chunks x 128
partitions x 8*129), so device DMAs are plain contiguous slices; the
activation engine turns each chunk into et = exp(beta - D) in one instruction.
Final: the two seam rows are DMA'd out and the join + log runs on the host in
float64.
"""

import numpy as np

B_TOT = 128          # total batch
N = 512              # rows (and cols) of each DP
M = 512
NCORES = 8
NB = B_TOT // NCORES  # 16 batches per core
H = N // 2           # 256 rows per direction
G = 4                # column groups per direction
W = M // G           # 128 columns per group
K = 8                # steps per DMA/act chunk
SLOT = W + 1         # 129: slot 0 = carry/halo passthrough column
TSTEPS = H + G - 1   # 259 macro-steps (t = 0..258)
NCHUNK = (TSTEPS + K - 1) // K  # 33
GAMMA = 1.0
BETA = -0.491        # drift rate: R_final/(N+M) for D ~ U[0,1], gamma=1
WB = float(np.exp(BETA))
BIG = 1.0e9          # staging slot-0 value that exp()s to 0 (border groups)

_CACHE = {}


def _build_bass():
    import concourse.bass as bass
    import concourse.mybir as mybir

    f32 = mybir.dt.float32
    nc = bass.Bass()

    dskew = nc.dram_tensor("Dskew", [NCHUNK, 128, K * SLOT], f32,
                           kind="ExternalInput")
    snap = nc.dram_tensor("snap", [G, 128, SLOT], f32, kind="ExternalOutput")

    from contextlib import ExitStack
    with ExitStack() as ctx:
        stg = [ctx.enter_context(nc.sbuf_tensor(f"stg{i}", [128, K * SLOT], f32))
               for i in range(3)]
        ering = [ctx.enter_context(nc.sbuf_tensor(f"er{i}", [128, K * SLOT], f32))
                 for i in range(3)]
        Bbuf = [ctx.enter_context(nc.sbuf_tensor(f"B{i}", [128, SLOT], f32))
                for i in range(2)]
        Abuf = [ctx.enter_context(nc.sbuf_tensor(f"A{i}", [128, SLOT], f32))
                for i in range(2)]
        carry = [ctx.enter_context(nc.sbuf_tensor(f"c{i}", [128, 1], f32))
                 for i in range(2)]
        bcol = ctx.enter_context(nc.sbuf_tensor("bcol", [128, 1], f32))

        with (
            nc.Block() as block,
            nc.semaphore("s_scan") as s_scan,
            nc.semaphore("s_stt") as s_stt,
            nc.semaphore("s_init") as s_init,
            nc.semaphore("s_carry") as s_carry,
            nc.semaphore("s_act") as s_act,
            nc.semaphore("s_dma0") as s_dma0,
            nc.semaphore("s_dma1") as s_dma1,
            nc.semaphore("s_dma2") as s_dma2,
            nc.semaphore("s_sn0") as s_sn0,
            nc.semaphore("s_sn1") as s_sn1,
            nc.semaphore("s_sn2") as s_sn2,
            nc.semaphore("s_sn3") as s_sn3,
        ):
            s_dma = [s_dma0, s_dma1, s_dma2]
            s_sn = [s_sn0, s_sn1, s_sn2, s_sn3]
            @block.sync
            def _(sp):
                for c in range(NCHUNK):
                    if c >= 3:
                        sp.wait_ge(s_act, c - 2)
                    src = bass.AP(dskew, c * 128 * K * SLOT,
                                  [[K * SLOT, 128], [1, K * SLOT]])
                    sp.dma_start(out=stg[c % 3][:, :], in_=src
                                 ).then_inc(s_dma[c % 3], 16)
                for g in range(G):
                    sp.wait_ge(s_scan, H + g)
                    dst = bass.AP(snap, g * 128 * SLOT, [[SLOT, 128], [1, SLOT]])
                    sp.dma_start(out=dst, in_=Bbuf[(H - 1 + g) % 2][:, :]
                                 ).then_inc(s_sn[g], 16)
                for g in range(G):
                    sp.wait_ge(s_sn[g], 16)

            @block.scalar
            def _(act):
                act.wait_ge(s_init, 8)  # prologue memsets done (bcol etc.)
                for c in range(NCHUNK):
                    act.wait_ge(s_dma[c % 3], 16 * (c // 3 + 1))
                    if c >= 3:
                        # ering slot c%3 last read by scans of chunk c-3
                        act.wait_ge(s_scan, 8 * (c - 3) + 8)
                    act.activation(
                        out=ering[c % 3][:, :], in_=stg[c % 3][:, :],
                        func=mybir.ActivationFunctionType.Exp,
                        bias=bcol[:, 0:1], scale=-1.0,
                    ).then_inc(s_act, 1)

            @block.gpsimd
            def _(gp):
                gp.wait_ge(s_init, 8)
                for t in range(TSTEPS):
                    if t > 0:
                        gp.wait_ge(s_scan, t)
                    bp = Bbuf[(t - 1) % 2]
                    # carry[p] <- B[p+32] in quadrant-legal pieces (SBUF APs
                    # must start at partition 0/32/64/96, counts <=128/32/64/32)
                    gp.tensor_copy(out=carry[t % 2][0:32, 0:1],
                                   in_=bp[32:64, W:SLOT]).then_inc(s_carry, 1)
                    gp.tensor_copy(out=carry[t % 2][32:64, 0:1],
                                   in_=bp[64:96, W:SLOT]).then_inc(s_carry, 1)
                    gp.tensor_copy(out=carry[t % 2][64:96, 0:1],
                                   in_=bp[96:128, W:SLOT]).then_inc(s_carry, 1)

            @block.vector
            def _(ve):
                # prologue: zero state, then plant the DP origin E(0,0)=1
                ve.memset(Bbuf[0][:, :], 0.0).then_inc(s_init, 1)
                ve.memset(Bbuf[1][:, 1:SLOT], 0.0).then_inc(s_init, 1)
                ve.memset(Bbuf[1][0:96, 0:1], 0.0).then_inc(s_init, 1)
                ve.memset(Abuf[0][:, :], 0.0).then_inc(s_init, 1)
                ve.memset(Abuf[1][:, :], 0.0).then_inc(s_init, 1)
                ve.memset(carry[0][:, :], 0.0).then_inc(s_init, 1)
                ve.memset(carry[1][:, :], 0.0).then_inc(s_init, 1)
                ve.memset(bcol[:, :], BETA).then_inc(s_init, 1)
                # origin: B1 slot0 = 1 on border-group partitions (p >= 96)
                ve.wait_ge(s_init, 8)
                ve.memset(Bbuf[1][96:128, 0:1], 1.0).then_inc(s_init, 1)
                for t in range(TSTEPS):
                    if t % 8 == 0:
                        ve.wait_ge(s_act, t // 8 + 1)
                    if t >= H + 1:          # snapshots guard B reuse (t=257,258)
                        ve.wait_ge(s_sn[t - H - 1], 16)
                    if t > 0:
                        ve.wait_ge(s_scan, t)   # B[prev] fully written
                    else:
                        ve.wait_ge(s_init, 9)   # origin memset done
                    bp = Bbuf[(t - 1) % 2]
                    bc = Bbuf[t % 2]
                    a = Abuf[t % 2]
                    ve.scalar_tensor_tensor(
                        out=a[:, 1:SLOT],
                        in0=bp[:, 0:W], scalar=WB, in1=bp[:, 1:SLOT],
                        op0=mybir.AluOpType.mult, op1=mybir.AluOpType.add,
                    ).then_inc(s_stt, 1)
                    ve.wait_ge(s_stt, t + 1)
                    ve.wait_ge(s_carry, 3 * (t + 1))
                    k = t % 8
                    ve.tensor_tensor_scan(
                        out=bc[:, :],
                        data0=a[:, :],
                        data1=ering[(t // 8) % 3][:, k * SLOT:(k + 1) * SLOT],
                        initial=carry[t % 2][:, 0:1],
                        op0=mybir.AluOpType.add, op1=mybir.AluOpType.mult,
                    ).then_inc(s_scan, 1)
                for g in range(G):
                    ve.wait_ge(s_sn[g], 16)

    return nc


def _make_dskew(Dc):
    """Dc: (NB, N, M) float32 -> (NCHUNK, 128, K*SLOT) staging layout."""
    Dtop = Dc[:, :H, :]
    Dbot = Dc[:, ::-1, ::-1][:, :H, :]
    out = np.zeros((NCHUNK, 128, K * SLOT), np.float32)
    p = np.arange(128)
    out[:, :, 0::SLOT] = np.where(p >= 96, BIG, BETA
                                  ).astype(np.float32)[None, :, None]

    TT = NCHUNK * K  # 264 step slots
    t = np.arange(TT)
    g = np.arange(G)
    rows = t[:, None] - g[None, :]              # (TT, G)
    valid = (rows >= 0) & (rows < H)
    rows_c = np.clip(rows, 0, H - 1)
    view = out.reshape(NCHUNK, 128, K, SLOT)[:, :, :, 1:]  # (33,128,8,128)
    for dirn, Dd in ((0, Dtop), (1, Dbot)):
        vals = np.empty((NB, TT, G, W), np.float32)
        for gg in range(G):
            vals[:, :, gg, :] = Dd[:, rows_c[:, gg], gg * W:(gg + 1) * W]
        vals *= valid[None, :, :, None]
        # (b, c, k, g, f) -> (c, g'(=3-g), b, k, f); p = g'*32 + dirn*16 + b
        arr = vals.reshape(NB, NCHUNK, K, G, W).transpose(1, 3, 0, 2, 4)
        arr = arr[:, ::-1, :, :, :]             # g -> g' = 3-g
        arr = arr.reshape(NCHUNK, G, NB, K, W)
        pview = view.reshape(NCHUNK, G, 2, NB, K, W)
        pview[:, :, dirn] = arr
    return out


def _combine(snap):
    """snap: (G, 128, SLOT) -> (NB,) float64 soft-DTW values."""
    Ff = np.zeros((NB, M + 1), np.float64)
    Fb = np.zeros((NB, M + 1), np.float64)
    b = np.arange(NB)
    for g in range(G):
        pf = (3 - g) * 32 + 0 * 16 + b
        pb = (3 - g) * 32 + 16 + b
        Ff[:, 1 + g * W: 1 + (g + 1) * W] = snap[g][pf, 1:]
        Fb[:, 1 + g * W: 1 + (g + 1) * W] = snap[g][pb, 1:]
    # E_total = e^{-(2H+M+1)b} * sum_k Ff[k] Fb[M+1-k]
    #         + e^{-(2H+M)b}   * sum_k Ff[k] Fb[M-k]
    S1 = np.sum(Ff[:, 1:] * Fb[:, 1:][:, ::-1], axis=1)
    S2 = np.sum(Ff[:, 1:] * Fb[:, 0:M][:, ::-1], axis=1)
    lt1 = np.log(S1) - BETA * (2 * H + M + 1)
    lt2 = np.log(S2) - BETA * (2 * H + M)
    mx = np.maximum(lt1, lt2)
    return -(mx + np.log(np.exp(lt1 - mx) + np.exp(lt2 - mx)))


def kernel(D):
    D = np.asarray(D, dtype=np.float32)
    assert D.shape == (B_TOT, N, M), D.shape
    if "nc" not in _CACHE:
        _CACHE["nc"] = _build_bass()
    nc = _CACHE["nc"]

    from concourse.bass_utils import run_bass_kernel_spmd
    in_maps = [{"Dskew": _make_dskew(D[c * NB:(c + 1) * NB])}
               for c in range(NCORES)]
    res = run_bass_kernel_spmd(nc, in_maps, list(range(NCORES)))
    outs = [_combine(res.results[c]["snap"]) for c in range(NCORES)]
    return np.concatenate(outs).astype(np.float32)


# revision 14
# speedup vs baseline: 1.4065x; 1.4065x over previous
"""Soft-DTW forward (gamma=1) Bass kernel for Trainium2, 8 NeuronCores.

Algorithm
---------
Exp-domain DP: with E = exp(-R/gamma), the soft-min recurrence becomes
    E[i,j] = exp(-D[i,j]) * (E[i-1,j] + E[i-1,j-1] + E[i,j-1])
(pure multiply-add, no per-cell exp/log).  Dynamic range is handled with a
drift factor:  F[i,j] = E[i,j] * exp(beta*(i+j)),  beta ~ E[dR/dstep], giving
    F[i,j] = et[i,j] * (F[i-1,j] + wb*F[i-1,j-1] + F[i,j-1])
with et = exp(beta - D), wb = exp(beta).  Per row this is a first-order linear
recurrence along j, which maps onto the DVE tensor_tensor_scan:
    state = (A_j + state) * et_j,   A_j = F_up[j] + wb*F_diag[j-1]
i.e. 2 DVE instructions per row-step (one scalar_tensor_tensor + one scan).

Parallelization: batch is sharded 16 per core (data parallel).  On a core, the
512x512 DP is split into fwd (rows 1..256 from the top-left) and bwd (rows
511..256 on the reversed matrix) halves, joined through a seam formula on the
two middle rows (soft-DTW path sums factor across the seam).  Each direction's
row is split into 4 column groups of width 128, software-pipelined in a
wavefront (group g processes row t-g at macro-step t), so every step is a
full 128-partition x 128-element instruction:
    partition p = b*8 + dir*4 + g        (b = batch-in-core, g = col group)
Cross-group carries move by a single DVE stream_shuffle (shift-by-1 inside
32-lane blocks) into the scan window's slot-0 column; the scan's
slot-0 "passthrough" column (A=0, et=1) regenerates each group's diagonal halo
without extra copies, and et's slot-0 value (0 for border groups) kills
garbage carries at column borders.

The host pre-skews D into the exact staging layout (33 chunks x 128
partitions x 8*129), so device DMAs are plain contiguous slices; the
activation engine turns each chunk into et = exp(beta - D) in one instruction.
Final: the two seam rows are DMA'd out and the join + log runs on the host in
float64.
"""

import numpy as np

B_TOT = 128          # total batch
N = 512              # rows (and cols) of each DP
M = 512
NCORES = 8
NB = B_TOT // NCORES  # 16 batches per core
H = N // 2           # 256 rows per direction
G = 4                # column groups per direction
W = M // G           # 128 columns per group
K = 8                # steps per DMA/act chunk
SLOT = W + 1         # 129: slot 0 = carry/halo passthrough column
TSTEPS = H + G - 1   # 259 macro-steps (t = 0..258)
NCHUNK = (TSTEPS + K - 1) // K  # 33
GAMMA = 1.0
BETA = -0.491        # drift rate: R_final/(N+M) for D ~ U[0,1], gamma=1
WB = float(np.exp(BETA))
BIG = 1.0e9          # staging slot-0 value that exp()s to 0 (border groups)
# stream_shuffle mask (32-lane blocks): out[k] = in[k-1] within each group of
# 4 lanes; k%4==0 keeps its own value (dead, killed by et slot-0 mask)
SHUF_MASK = [k if k % 4 == 0 else k - 1 for k in range(32)]

_CACHE = {}


def _build_bass():
    import concourse.bass as bass
    import concourse.mybir as mybir

    f32 = mybir.dt.float32
    nc = bass.Bass()

    dskew = nc.dram_tensor("Dskew", [NCHUNK, 128, K * SLOT], f32,
                           kind="ExternalInput")
    origin = nc.dram_tensor("origin", [128, 1], f32, kind="ExternalInput")
    snap = nc.dram_tensor("snap", [G, 128, SLOT], f32, kind="ExternalOutput")

    from contextlib import ExitStack
    with ExitStack() as ctx:
        stg = [ctx.enter_context(nc.sbuf_tensor(f"stg{i}", [128, K * SLOT], f32))
               for i in range(3)]
        ering = [ctx.enter_context(nc.sbuf_tensor(f"er{i}", [128, K * SLOT], f32))
                 for i in range(3)]
        Bbuf = [ctx.enter_context(nc.sbuf_tensor(f"B{i}", [128, SLOT], f32))
                for i in range(2)]
        Abuf = [ctx.enter_context(nc.sbuf_tensor(f"A{i}", [128, SLOT], f32))
                for i in range(2)]
        bcol = ctx.enter_context(nc.sbuf_tensor("bcol", [128, 1], f32))

        with (
            nc.Block() as block,
            nc.semaphore("s_scan") as s_scan,
            nc.semaphore("s_stt") as s_stt,
            nc.semaphore("s_init") as s_init,
            nc.semaphore("s_sh") as s_sh,
            nc.semaphore("s_act") as s_act,
            nc.semaphore("s_dma0") as s_dma0,
            nc.semaphore("s_dma1") as s_dma1,
            nc.semaphore("s_dma2") as s_dma2,
            nc.semaphore("s_sn0") as s_sn0,
            nc.semaphore("s_sn1") as s_sn1,
            nc.semaphore("s_sn2") as s_sn2,
            nc.semaphore("s_sn3") as s_sn3,
        ):
            s_dma = [s_dma0, s_dma1, s_dma2]
            s_sn = [s_sn0, s_sn1, s_sn2, s_sn3]
            @block.sync
            def _(sp):
                sp.wait_ge(s_init, 5)   # B1 memset done
                sp.dma_start(out=Bbuf[1][:, 0:1], in_=origin[:, :]
                             ).then_inc(s_sn0, 16)
                for c in range(NCHUNK):
                    if c >= 3:
                        sp.wait_ge(s_act, c - 2)
                    src = bass.AP(dskew, c * 128 * K * SLOT,
                                  [[K * SLOT, 128], [1, K * SLOT]])
                    sp.dma_start(out=stg[c % 3][:, :], in_=src
                                 ).then_inc(s_dma[c % 3], 16)
                for g in range(G):
                    sp.wait_ge(s_scan, H + g)
                    dst = bass.AP(snap, g * 128 * SLOT, [[SLOT, 128], [1, SLOT]])
                    sp.dma_start(out=dst, in_=Bbuf[(H - 1 + g) % 2][:, :]
                                 ).then_inc(s_sn[g], 16)
                for g in range(G):
                    sp.wait_ge(s_sn[g], 16)

            @block.scalar
            def _(act):
                act.wait_ge(s_init, 5)  # prologue memsets done (bcol etc.)
                for c in range(NCHUNK):
                    act.wait_ge(s_dma[c % 3], 16 * (c // 3 + 1))
                    if c >= 3:
                        # ering slot c%3 last read by scans of chunk c-3
                        act.wait_ge(s_scan, 8 * (c - 3) + 8)
                    act.activation(
                        out=ering[c % 3][:, :], in_=stg[c % 3][:, :],
                        func=mybir.ActivationFunctionType.Exp,
                        bias=bcol[:, 0:1], scale=-1.0,
                    ).then_inc(s_act, 1)

            @block.vector
            def _(ve):
                # prologue: zero state, then plant the DP origin E(0,0)=1
                ve.memset(Bbuf[0][:, :], 0.0).then_inc(s_init, 1)
                ve.memset(Bbuf[1][:, :], 0.0).then_inc(s_init, 1)
                ve.memset(Abuf[0][:, :], 0.0).then_inc(s_init, 1)
                ve.memset(Abuf[1][:, :], 0.0).then_inc(s_init, 1)
                ve.memset(bcol[:, :], BETA).then_inc(s_init, 1)
                # origin column (1.0 at p%4==0) DMA'd over B1 slot 0 by SP
                for t in range(TSTEPS):
                    if t % 8 == 0:
                        ve.wait_ge(s_act, t // 8 + 1)
                    if t >= H + 1:          # snapshots guard B reuse (t=257,258)
                        ve.wait_ge(s_sn[t - H - 1], 32 if t == H + 1 else 16)
                    if t > 0:
                        ve.wait_ge(s_scan, t)   # B[prev] fully written
                    else:
                        ve.wait_ge(s_sn0, 16)   # origin column DMA done
                    bp = Bbuf[(t - 1) % 2]
                    bc = Bbuf[t % 2]
                    a = Abuf[t % 2]
                    # carry: A[p,0] <- B_prev[p-1, W] for p%4 != 0 (group
                    # borders p%4==0 get garbage, killed by et slot0 = 0)
                    ve.stream_shuffle(
                        out=a[:, 0:1], in_=bp[:, W:SLOT], mask=SHUF_MASK,
                    ).then_inc(s_sh, 1)
                    ve.scalar_tensor_tensor(
                        out=a[:, 1:SLOT],
                        in0=bp[:, 0:W], scalar=WB, in1=bp[:, 1:SLOT],
                        op0=mybir.AluOpType.mult, op1=mybir.AluOpType.add,
                    ).then_inc(s_stt, 1)
                    ve.wait_ge(s_sh, t + 1)
                    ve.wait_ge(s_stt, t + 1)
                    k = t % 8
                    ve.tensor_tensor_scan(
                        out=bc[:, :],
                        data0=a[:, :],
                        data1=ering[(t // 8) % 3][:, k * SLOT:(k + 1) * SLOT],
                        initial=0.0,
                        op0=mybir.AluOpType.add, op1=mybir.AluOpType.mult,
                    ).then_inc(s_scan, 1)
                for g in range(G):
                    ve.wait_ge(s_sn[g], 16)

    return nc


def _make_dskew(Dc):
    """Dc: (NB, N, M) float32 -> (NCHUNK, 128, K*SLOT) staging layout."""
    Dtop = Dc[:, :H, :]
    Dbot = Dc[:, ::-1, ::-1][:, :H, :]
    out = np.zeros((NCHUNK, 128, K * SLOT), np.float32)
    p = np.arange(128)
    out[:, :, 0::SLOT] = np.where(p % 4 == 0, BIG, BETA
                                  ).astype(np.float32)[None, :, None]

    TT = NCHUNK * K  # 264 step slots
    t = np.arange(TT)
    g = np.arange(G)
    rows = t[:, None] - g[None, :]              # (TT, G)
    valid = (rows >= 0) & (rows < H)
    rows_c = np.clip(rows, 0, H - 1)
    view = out.reshape(NCHUNK, 128, K, SLOT)[:, :, :, 1:]  # (33,128,8,128)
    for dirn, Dd in ((0, Dtop), (1, Dbot)):
        vals = np.empty((NB, TT, G, W), np.float32)
        for gg in range(G):
            vals[:, :, gg, :] = Dd[:, rows_c[:, gg], gg * W:(gg + 1) * W]
        vals *= valid[None, :, :, None]
        # (b, c, k, g, f) -> (c, b, g, k, f); p = b*8 + dirn*4 + g
        arr = vals.reshape(NB, NCHUNK, K, G, W).transpose(1, 0, 3, 2, 4)
        pview = view.reshape(NCHUNK, NB, 2, G, K, W)
        pview[:, :, dirn] = arr
    return out


def _combine(snap):
    """snap: (G, 128, SLOT) -> (NB,) float64 soft-DTW values."""
    Ff = np.zeros((NB, M + 1), np.float64)
    Fb = np.zeros((NB, M + 1), np.float64)
    b = np.arange(NB)
    for g in range(G):
        pf = b * 8 + 0 * 4 + g
        pb = b * 8 + 4 + g
        Ff[:, 1 + g * W: 1 + (g + 1) * W] = snap[g][pf, 1:]
        Fb[:, 1 + g * W: 1 + (g + 1) * W] = snap[g][pb, 1:]
    # E_total = e^{-(2H+M+1)b} * sum_k Ff[k] Fb[M+1-k]
    #         + e^{-(2H+M)b}   * sum_k Ff[k] Fb[M-k]
    S1 = np.sum(Ff[:, 1:] * Fb[:, 1:][:, ::-1], axis=1)
    S2 = np.sum(Ff[:, 1:] * Fb[:, 0:M][:, ::-1], axis=1)
    lt1 = np.log(S1) - BETA * (2 * H + M + 1)
    lt2 = np.log(S2) - BETA * (2 * H + M)
    mx = np.maximum(lt1, lt2)
    return -(mx + np.log(np.exp(lt1 - mx) + np.exp(lt2 - mx)))


def kernel(D):
    D = np.asarray(D, dtype=np.float32)
    assert D.shape == (B_TOT, N, M), D.shape
    if "nc" not in _CACHE:
        _CACHE["nc"] = _build_bass()
    nc = _CACHE["nc"]

    from concourse.bass_utils import run_bass_kernel_spmd
    origin = (np.arange(128) % 4 == 0).astype(np.float32).reshape(128, 1)
    in_maps = [{"Dskew": _make_dskew(D[c * NB:(c + 1) * NB]),
                "origin": origin}
               for c in range(NCORES)]
    res = run_bass_kernel_spmd(nc, in_maps, list(range(NCORES)))
    outs = [_combine(res.results[c]["snap"]) for c in range(NCORES)]
    return np.concatenate(outs).astype(np.float32)


# revision 15
# speedup vs baseline: 2.3539x; 1.6735x over previous
"""Soft-DTW forward (gamma=1) Bass kernel for Trainium2, 8 NeuronCores.

Algorithm
---------
Exp-domain DP: with E = exp(-R/gamma), the soft-min recurrence becomes
    E[i,j] = exp(-D[i,j]) * (E[i-1,j] + E[i-1,j-1] + E[i,j-1])
(pure multiply-add, no per-cell exp/log).  Dynamic range is handled with a
drift factor:  F[i,j] = E[i,j] * exp(beta*(i+j)),  beta ~ E[dR/dstep], giving
    F[i,j] = et[i,j] * (F[i-1,j] + wb*F[i-1,j-1] + F[i,j-1])
with et = exp(beta - D), wb = exp(beta).  Per row this is a first-order linear
recurrence along j, which maps onto the DVE tensor_tensor_scan:
    state = (A_j + state) * et_j,   A_j = F_up[j] + wb*F_diag[j-1]
i.e. 2 DVE instructions per row-step (one scalar_tensor_tensor + one scan).

Parallelization: batch is sharded 16 per core (data parallel).  On a core, the
512x512 DP is split into fwd (rows 1..256 from the top-left) and bwd (rows
511..256 on the reversed matrix) halves, joined through a seam formula on the
two middle rows (soft-DTW path sums factor across the seam).  Each direction's
row is split into 4 column groups of width 128, software-pipelined in a
wavefront (group g processes row t-g at macro-step t), so every step is a
full 128-partition x 128-element instruction:
    partition p = b*8 + dir*4 + g        (b = batch-in-core, g = col group)
Cross-group carries move by a single DVE stream_shuffle (shift-by-1 inside
32-lane blocks) into the scan window's slot-0 column; the scan's
slot-0 "passthrough" column (A=0, et=1) regenerates each group's diagonal halo
without extra copies, and et's slot-0 value (0 for border groups) kills
garbage carries at column borders.

The host pre-skews D into the exact staging layout (33 chunks x 128
partitions x 8*129), so device DMAs are plain contiguous slices; the
activation engine turns each chunk into et = exp(beta - D) in one instruction.
Final: the two seam rows are DMA'd out and the join + log runs on the host in
float64.
"""

import numpy as np

B_TOT = 128          # total batch
N = 512              # rows (and cols) of each DP
M = 512
NCORES = 8
NB = B_TOT // NCORES  # 16 batches per core
H = N // 2           # 256 rows per direction
G = 4                # column groups per direction
W = M // G           # 128 columns per group
K = 8                # steps per DMA/act chunk
SLOT = W + 1         # 129: slot 0 = carry/halo passthrough column
TSTEPS = H + G - 1   # 259 macro-steps (t = 0..258)
SAFE_SYNC = False    # explicit sems between same-engine dependent DVE ops
NCHUNK = (TSTEPS + K - 1) // K  # 33
GAMMA = 1.0
BETA = -0.491        # drift rate: R_final/(N+M) for D ~ U[0,1], gamma=1
WB = float(np.exp(BETA))
BIG = 1.0e9          # staging slot-0 value that exp()s to 0 (border groups)
# stream_shuffle mask (32-lane blocks): out[k] = in[k-1] within each group of
# 4 lanes; k%4==0 keeps its own value (dead, killed by et slot-0 mask)
SHUF_MASK = [k if k % 4 == 0 else k - 1 for k in range(32)]

_CACHE = {}


def _build_bass():
    import concourse.bass as bass
    import concourse.mybir as mybir

    f32 = mybir.dt.float32
    nc = bass.Bass()

    dskew = nc.dram_tensor("Dskew", [NCHUNK, 128, K * SLOT], f32,
                           kind="ExternalInput")
    origin = nc.dram_tensor("origin", [128, 1], f32, kind="ExternalInput")
    snap = nc.dram_tensor("snap", [G, 128, SLOT], f32, kind="ExternalOutput")

    from contextlib import ExitStack
    with ExitStack() as ctx:
        stg = [ctx.enter_context(nc.sbuf_tensor(f"stg{i}", [128, K * SLOT], f32))
               for i in range(3)]
        ering = [ctx.enter_context(nc.sbuf_tensor(f"er{i}", [128, K * SLOT], f32))
                 for i in range(3)]
        Bbuf = [ctx.enter_context(nc.sbuf_tensor(f"B{i}", [128, SLOT], f32))
                for i in range(2)]
        Abuf = [ctx.enter_context(nc.sbuf_tensor(f"A{i}", [128, SLOT], f32))
                for i in range(2)]
        bcol = ctx.enter_context(nc.sbuf_tensor("bcol", [128, 1], f32))

        with (
            nc.Block() as block,
            nc.semaphore("s_scan") as s_scan,
            nc.semaphore("s_stt") as s_stt,
            nc.semaphore("s_init") as s_init,
            nc.semaphore("s_sh") as s_sh,
            nc.semaphore("s_act") as s_act,
            nc.semaphore("s_dma0") as s_dma0,
            nc.semaphore("s_dma1") as s_dma1,
            nc.semaphore("s_dma2") as s_dma2,
            nc.semaphore("s_sn0") as s_sn0,
            nc.semaphore("s_sn1") as s_sn1,
            nc.semaphore("s_sn2") as s_sn2,
            nc.semaphore("s_sn3") as s_sn3,
        ):
            s_dma = [s_dma0, s_dma1, s_dma2]
            s_sn = [s_sn0, s_sn1, s_sn2, s_sn3]
            @block.sync
            def _(sp):
                sp.wait_ge(s_init, 5)   # B1 memset done
                sp.dma_start(out=Bbuf[1][:, 0:1], in_=origin[:, :]
                             ).then_inc(s_sn0, 16)
                for c in range(NCHUNK):
                    if c >= 3:
                        sp.wait_ge(s_act, c - 2)
                    src = bass.AP(dskew, c * 128 * K * SLOT,
                                  [[K * SLOT, 128], [1, K * SLOT]])
                    sp.dma_start(out=stg[c % 3][:, :], in_=src
                                 ).then_inc(s_dma[c % 3], 16)
                for g in range(G):
                    sp.wait_ge(s_scan, H + g)
                    dst = bass.AP(snap, g * 128 * SLOT, [[SLOT, 128], [1, SLOT]])
                    sp.dma_start(out=dst, in_=Bbuf[(H - 1 + g) % 2][:, :]
                                 ).then_inc(s_sn[g], 16)
                for g in range(G):
                    sp.wait_ge(s_sn[g], 16)

            @block.scalar
            def _(act):
                act.wait_ge(s_init, 5)  # prologue memsets done (bcol etc.)
                for c in range(NCHUNK):
                    act.wait_ge(s_dma[c % 3], 16 * (c // 3 + 1))
                    if c >= 3:
                        # ering slot c%3 last read by scans of chunk c-3
                        act.wait_ge(s_scan, 8 * (c - 3) + 8)
                    act.activation(
                        out=ering[c % 3][:, :], in_=stg[c % 3][:, :],
                        func=mybir.ActivationFunctionType.Exp,
                        bias=bcol[:, 0:1], scale=-1.0,
                    ).then_inc(s_act, 1)

            @block.vector
            def _(ve):
                # prologue: zero state, then plant the DP origin E(0,0)=1
                ve.memset(Bbuf[0][:, :], 0.0).then_inc(s_init, 1)
                ve.memset(Bbuf[1][:, :], 0.0).then_inc(s_init, 1)
                ve.memset(Abuf[0][:, :], 0.0).then_inc(s_init, 1)
                ve.memset(Abuf[1][:, :], 0.0).then_inc(s_init, 1)
                ve.memset(bcol[:, :], BETA).then_inc(s_init, 1)
                # origin column (1.0 at p%4==0) DMA'd over B1 slot 0 by SP
                for t in range(TSTEPS):
                    if t % 8 == 0:
                        ve.wait_ge(s_act, t // 8 + 1)
                    if t >= H + 1:          # snapshots guard B reuse (t=257,258)
                        ve.wait_ge(s_sn[t - H - 1], 32 if t == H + 1 else 16)
                    if t > 0:
                        if SAFE_SYNC:
                            ve.wait_ge(s_scan, t)   # B[prev] fully written
                    else:
                        ve.wait_ge(s_sn0, 16)   # origin column DMA done
                    bp = Bbuf[(t - 1) % 2]
                    bc = Bbuf[t % 2]
                    a = Abuf[t % 2]
                    # carry: A[p,0] <- B_prev[p-1, W] for p%4 != 0 (group
                    # borders p%4==0 get garbage, killed by et slot0 = 0)
                    sh = ve.stream_shuffle(
                        out=a[:, 0:1], in_=bp[:, W:SLOT], mask=SHUF_MASK,
                    )
                    if SAFE_SYNC:
                        sh.then_inc(s_sh, 1)
                    st = ve.scalar_tensor_tensor(
                        out=a[:, 1:SLOT],
                        in0=bp[:, 0:W], scalar=WB, in1=bp[:, 1:SLOT],
                        op0=mybir.AluOpType.mult, op1=mybir.AluOpType.add,
                    )
                    if SAFE_SYNC:
                        st.then_inc(s_stt, 1)
                        ve.wait_ge(s_sh, t + 1)
                        ve.wait_ge(s_stt, t + 1)
                    k = t % 8
                    ve.tensor_tensor_scan(
                        out=bc[:, :],
                        data0=a[:, :],
                        data1=ering[(t // 8) % 3][:, k * SLOT:(k + 1) * SLOT],
                        initial=0.0,
                        op0=mybir.AluOpType.add, op1=mybir.AluOpType.mult,
                    ).then_inc(s_scan, 1)
                for g in range(G):
                    ve.wait_ge(s_sn[g], 16)

    return nc


def _make_dskew(Dc):
    """Dc: (NB, N, M) float32 -> (NCHUNK, 128, K*SLOT) staging layout."""
    Dtop = Dc[:, :H, :]
    Dbot = Dc[:, ::-1, ::-1][:, :H, :]
    out = np.zeros((NCHUNK, 128, K * SLOT), np.float32)
    p = np.arange(128)
    out[:, :, 0::SLOT] = np.where(p % 4 == 0, BIG, BETA
                                  ).astype(np.float32)[None, :, None]

    TT = NCHUNK * K  # 264 step slots
    t = np.arange(TT)
    g = np.arange(G)
    rows = t[:, None] - g[None, :]              # (TT, G)
    valid = (rows >= 0) & (rows < H)
    rows_c = np.clip(rows, 0, H - 1)
    view = out.reshape(NCHUNK, 128, K, SLOT)[:, :, :, 1:]  # (33,128,8,128)
    for dirn, Dd in ((0, Dtop), (1, Dbot)):
        vals = np.empty((NB, TT, G, W), np.float32)
        for gg in range(G):
            vals[:, :, gg, :] = Dd[:, rows_c[:, gg], gg * W:(gg + 1) * W]
        vals *= valid[None, :, :, None]
        # (b, c, k, g, f) -> (c, b, g, k, f); p = b*8 + dirn*4 + g
        arr = vals.reshape(NB, NCHUNK, K, G, W).transpose(1, 0, 3, 2, 4)
        pview = view.reshape(NCHUNK, NB, 2, G, K, W)
        pview[:, :, dirn] = arr
    return out


def _combine(snap):
    """snap: (G, 128, SLOT) -> (NB,) float64 soft-DTW values."""
    Ff = np.zeros((NB, M + 1), np.float64)
    Fb = np.zeros((NB, M + 1), np.float64)
    b = np.arange(NB)
    for g in range(G):
        pf = b * 8 + 0 * 4 + g
        pb = b * 8 + 4 + g
        Ff[:, 1 + g * W: 1 + (g + 1) * W] = snap[g][pf, 1:]
        Fb[:, 1 + g * W: 1 + (g + 1) * W] = snap[g][pb, 1:]
    # E_total = e^{-(2H+M+1)b} * sum_k Ff[k] Fb[M+1-k]
    #         + e^{-(2H+M)b}   * sum_k Ff[k] Fb[M-k]
    S1 = np.sum(Ff[:, 1:] * Fb[:, 1:][:, ::-1], axis=1)
    S2 = np.sum(Ff[:, 1:] * Fb[:, 0:M][:, ::-1], axis=1)
    lt1 = np.log(S1) - BETA * (2 * H + M + 1)
    lt2 = np.log(S2) - BETA * (2 * H + M)
    mx = np.maximum(lt1, lt2)
    return -(mx + np.log(np.exp(lt1 - mx) + np.exp(lt2 - mx)))


def kernel(D):
    D = np.asarray(D, dtype=np.float32)
    assert D.shape == (B_TOT, N, M), D.shape
    if "nc" not in _CACHE:
        _CACHE["nc"] = _build_bass()
    nc = _CACHE["nc"]

    from concourse.bass_utils import run_bass_kernel_spmd
    origin = (np.arange(128) % 4 == 0).astype(np.float32).reshape(128, 1)
    in_maps = [{"Dskew": _make_dskew(D[c * NB:(c + 1) * NB]),
                "origin": origin}
               for c in range(NCORES)]
    res = run_bass_kernel_spmd(nc, in_maps, list(range(NCORES)))
    outs = [_combine(res.results[c]["snap"]) for c in range(NCORES)]
    return np.concatenate(outs).astype(np.float32)
